# revision 17
# baseline (speedup 1.0000x reference)
"""DoubleGCN Trainium2 kernel v2: 8 NeuronCores, relation x node-half sharding.

Design vs v1 baseline:
- bf16 tables/payloads/S (halves gather DMA, enables FWL on matmuls)
- self-loops folded into the edge stream (no separate self-loop path)
- H1 computes the FULL h1*dinv table locally (features pre-transposed on
  host) -> no L1 AllGather; table written node-major directly
- dinv from host (index-derived metadata, like the bucketing itself)
- shared variable per-window chunk counts (max across cores) instead of a
  global fixed K
- TTR-fused psum evacuation + BN stats; single-shot BN apply
- gather calls sized NG=8192; S supertiles built 32 chunks at a time
"""
import numpy as np


class Cfg:
    def __init__(self, N, F, H, O, R, E, B, WIN=256, NG=8192, BPAD=128):
        assert N % 2 == 0
        self.N, self.F, self.H, self.O, self.R, self.E, self.B = N, F, H, O, R, E, B
        self.NHALF = N // 2
        self.WIN = WIN
        self.BANKW = 512 // WIN
        bank_nodes = WIN * self.BANKW          # 512
        self.NP = ((self.NHALF + bank_nodes - 1) // bank_nodes) * bank_nodes
        self.NW = self.NP // WIN
        self.NBANK = self.NW // self.BANKW
        self.NPF = 2 * self.NP                 # full padded table rows
        self.NBLKF = self.NPF // 128
        self.NBLK = self.NP // 128
        self.NG = NG
        self.OP = 64
        self.BP = ((B + BPAD - 1) // BPAD) * BPAD + BPAD
        # set after scanning data:
        self.counts = None    # tuple of (k_w0, k_w1) per window, shared across cores
        self.CH = None        # total chunks
        self.LS = None        # (L0, L1) padded gather-seq lengths per stream


def wrap_idx(idx_flat):
    L = idx_flat.shape[0]
    assert L % 16 == 0
    w = idx_flat.reshape(L // 16, 16).T
    return np.tile(w, (8, 1)).astype(np.int16)


def edge_lists_core(cfg, srcg, dstg, h):
    """Per-window, per-stream edge lists for one core, self-loops included.

    Returns wlists[w][s] = (src_local int array, dst_rel int array)."""
    NH, WIN, NW = cfg.NHALF, cfg.WIN, cfg.NW
    m = (dstg >= NH) == (h == 1)
    dl = (dstg[m] - h * NH).astype(np.int64)
    sg = srcg[m].astype(np.int64)
    # self loops: src_global = h*NH + d, dst_local = d
    selfd = np.arange(NH, dtype=np.int64)
    dl = np.concatenate([dl, selfd])
    sg = np.concatenate([sg, selfd + h * NH])
    order = np.argsort(dl, kind="stable")
    dl, sg = dl[order], sg[order]
    starts = np.searchsorted(dl, np.arange(NW) * WIN)
    ends = np.searchsorted(dl, np.arange(1, NW + 1) * WIN)
    wlists = []
    for w in range(NW):
        lo, hi = int(starts[w]), int(ends[w])
        swin, dwin = sg[lo:hi], dl[lo:hi] - w * WIN
        ma = swin < NH
        wlists.append((
            (swin[ma], dwin[ma]),
            (swin[~ma] - NH, dwin[~ma]),
        ))
    return wlists


def shared_counts(cfg, all_wlists):
    """Per (window, stream) chunk count = max over cores; stream0 >= 1."""
    NW = cfg.NW
    counts = []
    for w in range(NW):
        ks = [0, 0]
        for wl in all_wlists:
            for s in (0, 1):
                c = len(wl[w][s][0])
                ks[s] = max(ks[s], (c + 127) // 128)
        ks[0] = max(ks[0], 1)   # ensure every window writes its psum columns
        counts.append((ks[0], ks[1]))
    cfg.counts = tuple(counts)
    cfg.CH = sum(a + b for a, b in counts)
    ls = []
    for s in (0, 1):
        slots = 128 * sum(c[s] for c in counts)
        ls.append(((slots + cfg.NG - 1) // cfg.NG) * cfg.NG)
    cfg.LS = tuple(ls)


def pack_core(cfg, wlists):
    """Pack one core's edges into the shared chunk layout.

    Returns gidx0, gidx1 (wrapped int16), drel [128, CH] f32 (-1 pad)."""
    NW, CH = cfg.NW, cfg.CH
    drel = np.full((CH, 128), -1.0, np.float32)
    # trailing (beyond the last packed slot) = -1: the gather ucode trims
    # trailing negative indices, skipping their descriptors entirely
    seqs = [np.zeros(cfg.LS[0], np.int64), np.zeros(cfg.LS[1], np.int64)]
    pos = [0, 0]
    ci = 0
    for w in range(NW):
        for s in (0, 1):
            k = cfg.counts[w][s]
            si, di = wlists[w][s]
            n = len(si)
            assert n <= k * 128
            seqs[s][pos[s]:pos[s] + n] = si
            for j in range(k):
                seg = di[j * 128:(j + 1) * 128]
                if len(seg):
                    drel[ci + j, :len(seg)] = seg
            pos[s] += k * 128
            ci += k
    assert ci == CH
    return (wrap_idx(seqs[0].astype(np.int16)),
            wrap_idx(seqs[1].astype(np.int16)),
            drel.T.copy())


def prep_all(cfg, inputs):
    feats = np.asarray(inputs["features"], np.float32)
    edges = np.asarray(inputs["multi_r_edge_index"])
    batch = np.asarray(inputs["batch_nodes"])
    W1 = np.asarray(inputs["W1"], np.float32)
    g1 = np.asarray(inputs["g1"], np.float32)
    be1 = np.asarray(inputs["be1"], np.float32)
    W2 = np.asarray(inputs["W2"], np.float32)
    g2 = np.asarray(inputs["g2"], np.float32)
    be2 = np.asarray(inputs["be2"], np.float32)

    NH, NP, NPF, F, H, O, OP = (cfg.NHALF, cfg.NP, cfg.NPF, cfg.F, cfg.H,
                                cfg.O, cfg.OP)

    all_wlists = []
    for c in range(2 * cfg.R):
        r, h = c // 2, c % 2
        all_wlists.append(edge_lists_core(cfg, edges[r][0], edges[r][1], h))
    shared_counts(cfg, all_wlists)

    # full features, half-padded global layout, transposed: [128, F//128, NPF]
    ftab = np.zeros((NPF, F), np.float32)
    ftab[0:NH] = feats[0:NH]
    ftab[NP:NP + NH] = feats[NH:]
    fTt = ftab.T.reshape(F // 128, 128, NPF).transpose(1, 0, 2)
    fT = np.ascontiguousarray(
        fTt.reshape(128, F // 128, NPF // 512, 512).transpose(2, 0, 1, 3))

    iota = np.tile(np.arange(cfg.WIN, dtype=np.float32), (128, 1))
    ones_row = np.ones((1, 128), np.float32)

    cores = []
    for c in range(2 * cfg.R):
        r, h = c // 2, c % 2
        # per-core degree (own dst half, incl self-loop) -> dinv
        deg = np.zeros(NP, np.float64)
        wl = all_wlists[c]
        for w in range(cfg.NW):
            for s in (0, 1):
                di = wl[w][s][1]
                if len(di):
                    np.add.at(deg, w * cfg.WIN + di, 1.0)
        dinv_own = np.where(deg > 0, 1.0 / np.sqrt(deg), 0.0).astype(np.float32)
        cores.append(dict(h=h, r=r, dinv_own=dinv_own, wl=wl))

    # full dinv per relation (both halves) for the H1 table scale
    for r in range(cfg.R):
        full = np.zeros(NPF, np.float32)
        full[0:NP] = cores[2 * r]["dinv_own"]
        full[NP:] = cores[2 * r + 1]["dinv_own"]
        for h in (0, 1):
            cores[2 * r + h]["dinv_full"] = full

    out_cores = []
    for c in range(2 * cfg.R):
        r, h = c // 2, c % 2
        cd = cores[c]
        gidx0, gidx1, drel = pack_core(cfg, cd["wl"])
        # batch split
        bm = (batch >= NH) == (h == 1)
        pos = np.nonzero(bm)[0]
        bl = (batch[bm] - h * NH).astype(np.int64)
        bidx = np.zeros(cfg.BP, np.int64)
        bidx[:len(bl)] = bl
        W1p = W1[r].reshape(F // 128, 128, H).transpose(1, 0, 2).copy()  # [128, F//128, H]
        W2p = np.zeros((H, 2 * OP), np.float32)
        W2p[:, :O] = W2[r]
        bn2g = np.zeros((OP, 1), np.float32); bn2g[:O, 0] = g2[r]
        bn2b = np.zeros((OP, 1), np.float32); bn2b[:O, 0] = be2[r]
        dinv_colF = cd["dinv_full"].reshape(cfg.NBLKF, 128).T.copy()   # [128, NBLKF]
        out_cores.append(dict(
            tensors=dict(
                fT=fT,                      # cast bf16 at feed
                W1=W1p,                     # cast bf16
                W2p=W2p,                    # cast bf16
                bn1g=g1[r].reshape(H, 1).copy(), bn1b=be1[r].reshape(H, 1).copy(),
                bn2g=bn2g, bn2b=bn2b,
                gidx0=gidx0, gidx1=gidx1,
                drel=drel,                  # cast bf16
                bidx=wrap_idx(bidx.astype(np.int16)),
                iota=iota,                  # cast bf16
                dinv_colF=dinv_colF,        # f32 [128, NBLKF]
                dinv_row=cd["dinv_own"].reshape(1, NP).copy(),  # f32
                ones_row=ones_row,
            ),
            pos=pos, nb=len(bl),
        ))
    return out_cores


import concourse.bass as bass  # noqa: E402,F401
import concourse.mybir as mybir  # noqa: E402
import concourse.tile as tile  # noqa: E402
import concourse.bacc as bacc  # noqa: E402

f32 = mybir.dt.float32
bf16 = mybir.dt.bfloat16
i16 = mybir.dt.int16
AF = mybir.ActivationFunctionType
ALU = mybir.AluOpType
EPS = 1e-5
SG = 16          # chunks per S supertile
NUM_Q = 3        # SWDGE queues (rotated per gather supertile)
SP = False       # single_packet for gathers


def build(cfg, n_cores=8):
    N, F, H, O, OP = cfg.N, cfg.F, cfg.H, cfg.O, cfg.OP
    NP, NPF, NW, WIN, BP = cfg.NP, cfg.NPF, cfg.NW, cfg.WIN, cfg.BP
    NBANK, BANKW, NBLK, NBLKF = cfg.NBANK, cfg.BANKW, cfg.NBLK, cfg.NBLKF
    NG, CH, LS = cfg.NG, cfg.CH, cfg.LS
    counts = cfg.counts
    CPC = NG // 128
    PAIRS = [[2 * i, 2 * i + 1] for i in range(max(1, n_cores // 2))]

    nc = bacc.Bacc(None, target_bir_lowering=False,
                   num_swdge_queues=NUM_Q)

    fTd = nc.dram_tensor("fT", [NPF // 512, 128, F // 128, 512], bf16, kind="ExternalInput")
    W1d = nc.dram_tensor("W1", [128, F // 128, H], bf16, kind="ExternalInput")
    W2d = nc.dram_tensor("W2p", [H, 2 * OP], bf16, kind="ExternalInput")
    bn1g = nc.dram_tensor("bn1g", [H, 1], f32, kind="ExternalInput")
    bn1b = nc.dram_tensor("bn1b", [H, 1], f32, kind="ExternalInput")
    bn2g = nc.dram_tensor("bn2g", [OP, 1], f32, kind="ExternalInput")
    bn2b = nc.dram_tensor("bn2b", [OP, 1], f32, kind="ExternalInput")
    gidxd = [nc.dram_tensor("gidx0", [128, LS[0] // 16], i16, kind="ExternalInput"),
             nc.dram_tensor("gidx1", [128, LS[1] // 16], i16, kind="ExternalInput")]
    dreld = nc.dram_tensor("drel", [128, CH], bf16, kind="ExternalInput")
    iotad = nc.dram_tensor("iota", [128, WIN], bf16, kind="ExternalInput")
    bidxd = nc.dram_tensor("bidx", [128, BP // 16], i16, kind="ExternalInput")
    dcolFd = nc.dram_tensor("dinv_colF", [128, NBLKF], f32, kind="ExternalInput")
    dcolOd = nc.dram_tensor("dinv_colO", [128, NBLK], f32, kind="ExternalInput")
    dpatd = nc.dram_tensor("dinv_pat", [128, NP], f32, kind="ExternalInput")
    identd = nc.dram_tensor("ident", [128, 128], f32, kind="ExternalInput")
    onesrd = nc.dram_tensor("ones_row", [1, 128], f32, kind="ExternalInput")
    outd = nc.dram_tensor("out", [BP, OP], f32, kind="ExternalOutput")

    # per-window chunk walk metadata (shared across cores)
    # wchunks[w] = list of (s, global_ci)
    wchunks = []
    ci = 0
    for w in range(NW):
        lst = []
        for s in (0, 1):
            for _ in range(counts[w][s]):
                lst.append((s, ci))
                ci += 1
        wchunks.append(lst)
    assert ci == CH

    with tile.TileContext(nc) as tc:
        with (
            tc.tile_pool(name="const", bufs=1) as cp,
            tc.tile_pool(name="dram", bufs=1, space="DRAM") as dp,
            tc.tile_pool(name="gbufp", bufs=1) as gp,
        ):
            ident = cp.tile([128, 128], f32); nc.sync.dma_start(ident[:], identd[:])
            ones_row = cp.tile([1, 128], f32); nc.sync.dma_start(ones_row[:], onesrd[:])
            iota = cp.tile([128, WIN], bf16); nc.sync.dma_start(iota[:], iotad[:])
            drel = cp.tile([128, CH], bf16); nc.sync.dma_start(drel[:], dreld[:])
            W1sb = cp.tile([128, F // 128, H], bf16); nc.sync.dma_start(W1sb[:], W1d[:])
            W2sb = cp.tile([H, 2 * OP], bf16); nc.sync.dma_start(W2sb[:], W2d[:])
            dcolF = cp.tile([128, NBLKF], f32); nc.sync.dma_start(dcolF[:], dcolFd[:])
            dcolO = cp.tile([128, NBLK], f32); nc.sync.dma_start(dcolO[:], dcolOd[:])
            bn1gt = cp.tile([H, 1], f32); nc.sync.dma_start(bn1gt[:], bn1g[:])
            bn1bt = cp.tile([H, 1], f32); nc.sync.dma_start(bn1bt[:], bn1b[:])
            bn2gt = cp.tile([OP, 1], f32); nc.sync.dma_start(bn2gt[:], bn2g[:])
            bn2bt = cp.tile([OP, 1], f32); nc.sync.dma_start(bn2bt[:], bn2b[:])
            epst = cp.tile([128, 1], f32); nc.vector.memset(epst[:], EPS)
            st1 = cp.tile([H, 2 * NBANK], f32)
            st2 = cp.tile([OP, 2 * NBANK], f32)
            gbuf = gp.tile([128, NP], bf16)

            g1tab = dp.tile([NPF, H], bf16)
            g2half = dp.tile([NP, OP], f32)
            g2tab = dp.tile([2, NP, OP], f32)
            x2pre = dp.tile([NP, OP], f32)
            st1l = dp.tile([H, 2], f32)
            st1gl = dp.tile([H, 2], f32)
            st2l = dp.tile([OP, 2], f32)
            st2gl = dp.tile([OP, 2], f32)

            # ============ H1: full table h1*dinv, node-major ============
            with (
                tc.tile_pool(name="h1sb", bufs=4) as hp,
                tc.tile_pool(name="h1ps", bufs=4, space="PSUM") as hps,
            ):
                with nc.named_scope("h1"):
                    for g in range(NBLKF // 4):
                        ftile = hp.tile([128, F // 128, 512], bf16, tag="ftile")
                        nc.sync.dma_start(ftile[:], fTd[g])
                        h1p = hps.tile([128, 4, H], f32, tag="h1p")
                        for q in range(4):
                            for k in range(F // 128):
                                nc.tensor.matmul(
                                    h1p[:, q, :],
                                    ftile[:, k, 128 * q:128 * q + 128],
                                    W1sb[:, k, :],
                                    start=(k == 0), stop=(k == F // 128 - 1))
                        h1nm = hp.tile([128, 4, H], bf16, tag="h1nm")
                        for q in range(4):
                            nc.scalar.activation(
                                h1nm[:, q, :], h1p[:, q, :], AF.Copy,
                                scale=dcolF[:, 4 * g + q:4 * g + q + 1])
                        nc.sync.dma_start(
                            g1tab[512 * g:512 * g + 512, :].rearrange("(q p) h -> p q h", p=128),
                            h1nm[:])

            # ============ shared seg pass ============
            def seg_pass(scope, tab_views, elem, fin):
                with (
                    tc.tile_pool(name="segsb", bufs=4) as sp,
                    tc.tile_pool(name="finp", bufs=2) as fnp,
                    tc.tile_pool(name="gat", bufs=3) as gpp,
                    tc.tile_pool(name="idxp", bufs=2) as ip,
                    tc.tile_pool(name="patp", bufs=2) as pp2,
                    tc.tile_pool(name="psseg", bufs=3, space="PSUM") as pss,
                    tc.tile_pool(name="psfin", bufs=2, space="PSUM") as psf,
                ):
                    with nc.named_scope(scope):
                        pos = [0, 0]
                        cache = [dict(), dict()]
                        qctr = [0]
                        Ssup, g0 = None, -1

                        def payload(s):
                            q, slot = pos[s] // CPC, pos[s] % CPC
                            if q not in cache[s]:
                                idxt = ip.tile([128, NG // 16], i16, tag=f"idx{s}")
                                nc.sync.dma_start(
                                    idxt[:],
                                    gidxd[s][:, q * (NG // 16):(q + 1) * (NG // 16)])
                                gt = gpp.tile([128, CPC, elem], bf16, tag=f"gt{s}")
                                nc.gpsimd.dma_gather(
                                    gt[:], tab_views[s], idxt[:], NG, NG, elem,
                                    single_packet=SP, queue_num=qctr[0] % NUM_Q)
                                qctr[0] += 1
                                cache[s] = {q: gt}
                            pos[s] += 1
                            return cache[s][q][:, slot, :]

                        for b in range(NBANK):
                            segp = pss.tile([128, 512], f32, tag="segp")
                            patf = pp2.tile([128, 512], f32, tag="patf")
                            nc.sync.dma_start(patf[:], dpatd[:, 512 * b:512 * b + 512])
                            for j in range(BANKW):
                                w = b * BANKW + j
                                ck = wchunks[w]
                                for t, (s, ci) in enumerate(ck):
                                    if ci % SG == 0:
                                        g0 = ci
                                        Ssup = sp.tile([128, SG, WIN], bf16, tag="S")
                                        nsg = min(SG, CH - g0)
                                        nc.vector.tensor_tensor(
                                            Ssup[:, 0:nsg, :],
                                            drel[:, g0:g0 + nsg].unsqueeze(2)
                                                .broadcast_to([128, nsg, WIN]),
                                            iota[:].unsqueeze(1)
                                                .broadcast_to([128, nsg, WIN]),
                                            ALU.is_equal)
                                    pl = payload(s)
                                    nc.tensor.matmul(
                                        segp[:, WIN * j:WIN * j + WIN],
                                        pl, Ssup[:, ci - g0, :],
                                        start=(t == 0), stop=(t == len(ck) - 1))
                            fin(b, segp, patf, dict(sb=fnp, ps=psf))

            # ============ L1 fin ============
            def fin1(b, segp, patf, pools):
                sl = gbuf[:, 512 * b:512 * b + 512]
                nc.vector.tensor_tensor(sl, segp[:], patf[:], ALU.mult)
                nc.vector.tensor_reduce(
                    st1[:, 2 * b:2 * b + 1], sl, mybir.AxisListType.X, ALU.add)
                scr = pools["sb"].tile([128, 512], f32, tag="scr")
                nc.scalar.activation(scr[:], sl, AF.Square)
                nc.vector.tensor_reduce(
                    st1[:, 2 * b + 1:2 * b + 2], scr[:], mybir.AxisListType.X, ALU.add)

            seg_pass("seg1", [g1tab[0:NP], g1tab[NP:NPF]], H, fin1)

            # ============ BN1 ============
            def bn_block(st, n_bank, stl, stgl, gt, bt, P, scope):
                with tc.tile_pool(name=f"bn{scope}", bufs=1) as bp:
                    stv = bp.tile([P, 2], f32)
                    nc.vector.tensor_reduce(
                        stv[:, 0:1], st[:, 0:2 * n_bank:2], mybir.AxisListType.X, ALU.add)
                    nc.vector.tensor_reduce(
                        stv[:, 1:2], st[:, 1:2 * n_bank:2], mybir.AxisListType.X, ALU.add)
                    nc.sync.dma_start(stl[:], stv[:])
                    nc.gpsimd.collective_compute(
                        "AllReduce", ALU.add, replica_groups=PAIRS,
                        ins=[stl.opt()], outs=[stgl.opt()])
                    stg = bp.tile([P, 2], f32)
                    nc.sync.dma_start(stg[:], stgl[:])
                    mean = bp.tile([P, 1], f32)
                    nc.vector.tensor_scalar_mul(mean[:], stg[:, 0:1], 1.0 / N)
                    var = bp.tile([P, 1], f32)
                    nc.vector.tensor_scalar_mul(var[:], stg[:, 1:2], 1.0 / N)
                    msq = bp.tile([P, 1], f32)
                    nc.vector.tensor_tensor(msq[:], mean[:], mean[:], ALU.mult)
                    nc.vector.tensor_tensor(var[:], var[:], msq[:], ALU.subtract)
                    nc.scalar.activation(var[:], var[:], AF.Sqrt, bias=epst[0:P, :])
                    rstd = bp.tile([P, 1], f32)
                    nc.vector.reciprocal(rstd[:], var[:])
                    sc = bp.tile([P, 1], f32)
                    nc.vector.tensor_tensor(sc[:], gt[:], rstd[:], ALU.mult)
                    sh = bp.tile([P, 1], f32)
                    nc.vector.tensor_tensor(sh[:], mean[:], sc[:], ALU.mult)
                    nc.vector.tensor_tensor(sh[:], bt[:], sh[:], ALU.subtract)
                    return sc, sh

            with nc.named_scope("bn1"):
                sc1, sh1 = bn_block(st1, NBANK, st1l, st1gl, bn1gt, bn1bt, H, "1")
                nc.scalar.activation(gbuf[:], gbuf[:], AF.Relu, bias=sh1[:], scale=sc1[:])

            # ============ L2 table ============
            with (
                tc.tile_pool(name="l2sb", bufs=3) as lp,
                tc.tile_pool(name="l2ps", bufs=3, space="PSUM") as lps,
            ):
                with nc.named_scope("l2tab"):
                    for g in range(NBLK // 4):
                        h2p = lps.tile([128, 4, 2 * OP], f32, tag="h2p")
                        for q in range(4):
                            t = 4 * g + q
                            nc.tensor.matmul(
                                h2p[:, q, :], gbuf[:, 128 * t:128 * t + 128],
                                W2sb[:], start=True, stop=True)
                        h2nm = lp.tile([128, 4, 2 * OP], bf16, tag="h2nm")
                        for q in range(4):
                            nc.scalar.activation(
                                h2nm[:, q, :], h2p[:, q, :], AF.Copy,
                                scale=dcolO[:, 4 * g + q:4 * g + q + 1])
                        nc.sync.dma_start(
                            g2half[512 * g:512 * g + 512, :].rearrange("(q p) e -> p q e", p=128),
                            h2nm[:].bitcast(f32))

            nc.gpsimd.collective_compute(
                "AllGather", ALU.bypass, replica_groups=PAIRS,
                ins=[g2half.opt()], outs=[g2tab.opt()])

            # ============ L2 fin ============
            def fin2(b, segp, patf, pools):
                x2sl = pools["sb"].tile([64, 512], f32, tag="x2sl")
                nc.vector.tensor_tensor(x2sl[:], segp[0:64, :], patf[0:64, :], ALU.mult)
                nc.vector.tensor_reduce(
                    st2[:, 2 * b:2 * b + 1], x2sl[:], mybir.AxisListType.X, ALU.add)
                scr = pools["sb"].tile([64, 512], f32, tag="scr2")
                nc.scalar.activation(scr[:], x2sl[:], AF.Square)
                nc.vector.tensor_reduce(
                    st2[:, 2 * b + 1:2 * b + 2], scr[:], mybir.AxisListType.X, ALU.add)
                x2nm = pools["sb"].tile([128, 4, OP], f32, tag="x2nm")
                for q in range(4):
                    trp = pools["ps"].tile([128, 128], f32, tag="trp")
                    nc.tensor.transpose(trp[0:128, 0:64], x2sl[:, 128 * q:128 * q + 128],
                                        ident[0:64, 0:64])
                    nc.scalar.activation(x2nm[:, q, :], trp[0:128, 0:64], AF.Copy)
                nc.sync.dma_start(
                    x2pre[512 * b:512 * b + 512, :].rearrange("(q p) e -> p q e", p=128),
                    x2nm[:])

            seg_pass("seg2", [g2tab[0].bitcast(bf16), g2tab[1].bitcast(bf16)], 2 * OP, fin2)

            # ============ BN2 + FINAL ============
            with (
                tc.tile_pool(name="fsb", bufs=2) as fp2,
                tc.tile_pool(name="fps", bufs=2, space="PSUM") as pf,
            ):
                with nc.named_scope("final"):
                    bidxt0 = fp2.tile([128, BP // 16], i16, tag="bidx")
                    nc.sync.dma_start(bidxt0[:], bidxd[:])
                    NBB0 = BP // 128
                    xb0 = fp2.tile([128, NBB0, OP], f32, tag="xb")
                    nc.gpsimd.dma_gather(xb0[:], x2pre[:], bidxt0[:], BP, BP, OP,
                                         single_packet=False)
                    sc2, sh2 = bn_block(st2, NBANK, st2l, st2gl, bn2gt, bn2bt, OP, "2")
                    reps = []
                    for vi, v in enumerate((sc2, sh2)):
                        rowp = pf.tile([128, 128], f32, tag="rowp")
                        nc.tensor.matmul(rowp[0:1, 0:OP], v[:], ident[0:OP, 0:OP],
                                         start=True, stop=True)
                        rowsb = fp2.tile([1, OP], f32, tag=f"rowsb{vi}")
                        nc.scalar.activation(rowsb[:], rowp[0:1, 0:OP], AF.Copy)
                        repp = pf.tile([128, OP], f32, tag=f"rep{vi}")
                        nc.tensor.matmul(repp[:], ones_row[:], rowsb[:], start=True, stop=True)
                        rep = fp2.tile([128, OP], f32, tag=f"repsb{vi}")
                        nc.vector.tensor_copy(rep[:], repp[:])
                        reps.append(rep)
                    sc_rep, sh_rep = reps

                    NBB = NBB0
                    xb = xb0
                    nc.vector.tensor_tensor(
                        xb[:], xb[:], sc_rep[:].unsqueeze(1).broadcast_to([128, NBB, OP]),
                        ALU.mult)
                    nc.vector.tensor_tensor(
                        xb[:], xb[:], sh_rep[:].unsqueeze(1).broadcast_to([128, NBB, OP]),
                        ALU.add)
                    nc.scalar.activation(xb[:], xb[:], AF.Relu)
                    xs = xb[:, :, 0:O]
                    mx = fp2.tile([128, NBB, 1], f32, tag="mx")
                    nc.vector.tensor_reduce(mx[:], xs, mybir.AxisListType.X, ALU.max)
                    nc.vector.tensor_tensor(xs, xs, mx[:].broadcast_to([128, NBB, O]),
                                            ALU.subtract)
                    ex = fp2.tile([128, NBB, O], f32, tag="ex")
                    nc.scalar.activation(ex[:], xs, AF.Exp)
                    sm = fp2.tile([128, NBB, 1], f32, tag="sm")
                    nc.vector.tensor_reduce(sm[:], ex[:], mybir.AxisListType.X, ALU.add)
                    nc.scalar.activation(sm[:], sm[:], AF.Ln)
                    nc.vector.tensor_tensor(xs, xs, sm[:].broadcast_to([128, NBB, O]),
                                            ALU.subtract)
                    nc.sync.dma_start(outd[:].rearrange("(g p) e -> p g e", p=128), xb[:])

    nc.compile()
    return nc


import ml_dtypes  # noqa: E402

TRACE = False
LAST = {"exec_time_ns": None}
_CACHE = {}


def _get_program(cfg):
    key = (cfg.N, cfg.F, cfg.H, cfg.O, cfg.R, cfg.E, cfg.B, hash(cfg.counts))
    if key not in _CACHE:
        _CACHE[key] = build(cfg, n_cores=8)
    return _CACHE[key]


def kernel(**inputs):
    from concourse.bass_utils import run_bass_kernel_spmd

    inputs = {k: np.asarray(v) for k, v in inputs.items()}
    N, F = inputs["features"].shape
    R, _, E = inputs["multi_r_edge_index"].shape
    B = inputs["batch_nodes"].shape[0]
    H = inputs["W1"].shape[2]
    O = inputs["W2"].shape[2]
    cfg = Cfg(N=N, F=F, H=H, O=O, R=R, E=E, B=B)
    cores = prep_all(cfg, inputs)
    nc = _get_program(cfg)

    ident = np.eye(128, dtype=np.float32)

    in_maps = []
    for c in range(2 * R):
        t = cores[c]["tensors"]
        dpat = np.tile(t["dinv_row"], (128, 1)).astype(np.float32)
        in_maps.append(dict(
            fT=t["fT"].astype(ml_dtypes.bfloat16),
            W1=t["W1"].astype(ml_dtypes.bfloat16),
            W2p=t["W2p"].astype(ml_dtypes.bfloat16),
            bn1g=t["bn1g"], bn1b=t["bn1b"], bn2g=t["bn2g"], bn2b=t["bn2b"],
            gidx0=t["gidx0"], gidx1=t["gidx1"],
            drel=t["drel"].astype(ml_dtypes.bfloat16),
            bidx=t["bidx"],
            iota=t["iota"].astype(ml_dtypes.bfloat16),
            dinv_colF=t["dinv_colF"],
            dinv_colO=t["dinv_colF"][:, (c % 2) * cfg.NBLK:(c % 2 + 1) * cfg.NBLK].copy(),
            dinv_pat=dpat,
            ident=ident,
            ones_row=t["ones_row"],
        ))

    res = run_bass_kernel_spmd(nc, in_maps, core_ids=list(range(2 * R)), trace=TRACE)
    LAST["exec_time_ns"] = res.exec_time_ns
    LAST["results"] = res

    out = np.zeros((B, R * O), np.float32)
    for c in range(2 * R):
        core = cores[c]
        r = c // 2
        row = np.asarray(res.results[c]["out"])
        out[core["pos"], r * O:(r + 1) * O] = row[:core["nb"], :O]
    return out



# revision 18
# speedup vs baseline: 1.0166x; 1.0166x over previous
"""DoubleGCN Trainium2 kernel v2: 8 NeuronCores, relation x node-half sharding.

Design vs v1 baseline:
- bf16 tables/payloads/S (halves gather DMA, enables FWL on matmuls)
- self-loops folded into the edge stream (no separate self-loop path)
- H1 computes the FULL h1*dinv table locally (features pre-transposed on
  host) -> no L1 AllGather; table written node-major directly
- dinv from host (index-derived metadata, like the bucketing itself)
- shared variable per-window chunk counts (max across cores) instead of a
  global fixed K
- TTR-fused psum evacuation + BN stats; single-shot BN apply
- gather calls sized NG=8192; S supertiles built 32 chunks at a time
"""
import numpy as np


class Cfg:
    def __init__(self, N, F, H, O, R, E, B, WIN=256, NG=8192, BPAD=128):
        assert N % 2 == 0
        self.N, self.F, self.H, self.O, self.R, self.E, self.B = N, F, H, O, R, E, B
        self.NHALF = N // 2
        self.WIN = WIN
        self.BANKW = 512 // WIN
        bank_nodes = WIN * self.BANKW          # 512
        self.NP = ((self.NHALF + bank_nodes - 1) // bank_nodes) * bank_nodes
        self.NW = self.NP // WIN
        self.NBANK = self.NW // self.BANKW
        self.NPF = 2 * self.NP                 # full padded table rows
        self.NBLKF = self.NPF // 128
        self.NBLK = self.NP // 128
        self.NG = NG
        self.OP = 64
        self.BP = ((B + BPAD - 1) // BPAD) * BPAD + BPAD
        # set after scanning data:
        self.counts = None    # tuple of (k_w0, k_w1) per window, shared across cores
        self.CH = None        # total chunks
        self.LS = None        # (L0, L1) padded gather-seq lengths per stream


def rowperm(n):
    """Table-row permutation matching the (p q) block write layout:
    node n (within a half) lives at row blk*512 + p*4 + q."""
    blk = n // 512
    r = n % 512
    return blk * 512 + (r % 128) * 4 + (r // 128)


def wrap_idx(idx_flat):
    L = idx_flat.shape[0]
    assert L % 16 == 0
    w = idx_flat.reshape(L // 16, 16).T
    return np.tile(w, (8, 1)).astype(np.int16)


def edge_lists_core(cfg, srcg, dstg, h):
    """Per-window, per-stream edge lists for one core, self-loops included.

    Returns wlists[w][s] = (src_local int array, dst_rel int array)."""
    NH, WIN, NW = cfg.NHALF, cfg.WIN, cfg.NW
    m = (dstg >= NH) == (h == 1)
    dl = (dstg[m] - h * NH).astype(np.int64)
    sg = srcg[m].astype(np.int64)
    # self loops: src_global = h*NH + d, dst_local = d
    selfd = np.arange(NH, dtype=np.int64)
    dl = np.concatenate([dl, selfd])
    sg = np.concatenate([sg, selfd + h * NH])
    order = np.argsort(dl, kind="stable")
    dl, sg = dl[order], sg[order]
    starts = np.searchsorted(dl, np.arange(NW) * WIN)
    ends = np.searchsorted(dl, np.arange(1, NW + 1) * WIN)
    wlists = []
    for w in range(NW):
        lo, hi = int(starts[w]), int(ends[w])
        swin, dwin = sg[lo:hi], dl[lo:hi] - w * WIN
        ma = swin < NH
        wlists.append((
            (swin[ma], dwin[ma]),
            (swin[~ma] - NH, dwin[~ma]),
        ))
    return wlists


def shared_counts(cfg, all_wlists):
    """Per (window, stream) chunk count = max over cores; stream0 >= 1."""
    NW = cfg.NW
    counts = []
    for w in range(NW):
        ks = [0, 0]
        for wl in all_wlists:
            for s in (0, 1):
                c = len(wl[w][s][0])
                ks[s] = max(ks[s], (c + 127) // 128)
        ks[0] = max(ks[0], 1)   # ensure every window writes its psum columns
        counts.append((ks[0], ks[1]))
    cfg.counts = tuple(counts)
    cfg.CH = sum(a + b for a, b in counts)
    ls = []
    for s in (0, 1):
        slots = 128 * sum(c[s] for c in counts)
        ls.append(((slots + cfg.NG - 1) // cfg.NG) * cfg.NG)
    cfg.LS = tuple(ls)


def pack_core(cfg, wlists):
    """Pack one core's edges into the shared chunk layout.

    Returns gidx0, gidx1 (wrapped int16), drel [128, CH] f32 (-1 pad)."""
    NW, CH = cfg.NW, cfg.CH
    drel = np.full((CH, 128), -1.0, np.float32)
    # trailing (beyond the last packed slot) = -1: the gather ucode trims
    # trailing negative indices, skipping their descriptors entirely
    seqs = [np.zeros(cfg.LS[0], np.int64), np.zeros(cfg.LS[1], np.int64)]
    pos = [0, 0]
    ci = 0
    for w in range(NW):
        for s in (0, 1):
            k = cfg.counts[w][s]
            si, di = wlists[w][s]
            n = len(si)
            assert n <= k * 128
            seqs[s][pos[s]:pos[s] + n] = si
            for j in range(k):
                seg = di[j * 128:(j + 1) * 128]
                if len(seg):
                    drel[ci + j, :len(seg)] = seg
            pos[s] += k * 128
            ci += k
    assert ci == CH
    return (wrap_idx(rowperm(seqs[0]).astype(np.int16)),
            wrap_idx(rowperm(seqs[1]).astype(np.int16)),
            drel.T.copy())


def prep_all(cfg, inputs):
    feats = np.asarray(inputs["features"], np.float32)
    edges = np.asarray(inputs["multi_r_edge_index"])
    batch = np.asarray(inputs["batch_nodes"])
    W1 = np.asarray(inputs["W1"], np.float32)
    g1 = np.asarray(inputs["g1"], np.float32)
    be1 = np.asarray(inputs["be1"], np.float32)
    W2 = np.asarray(inputs["W2"], np.float32)
    g2 = np.asarray(inputs["g2"], np.float32)
    be2 = np.asarray(inputs["be2"], np.float32)

    NH, NP, NPF, F, H, O, OP = (cfg.NHALF, cfg.NP, cfg.NPF, cfg.F, cfg.H,
                                cfg.O, cfg.OP)

    all_wlists = []
    for c in range(2 * cfg.R):
        r, h = c // 2, c % 2
        all_wlists.append(edge_lists_core(cfg, edges[r][0], edges[r][1], h))
    shared_counts(cfg, all_wlists)

    # full features, half-padded global layout, transposed: [128, F//128, NPF]
    ftab = np.zeros((NPF, F), np.float32)
    ftab[0:NH] = feats[0:NH]
    ftab[NP:NP + NH] = feats[NH:]
    fTt = ftab.T.reshape(F // 128, 128, NPF).transpose(1, 0, 2)
    fT = np.ascontiguousarray(
        fTt.reshape(128, F // 128, NPF // 512, 512).transpose(2, 0, 1, 3))

    iota = np.tile(np.arange(cfg.WIN, dtype=np.float32), (128, 1))
    ones_row = np.ones((1, 128), np.float32)

    cores = []
    for c in range(2 * cfg.R):
        r, h = c // 2, c % 2
        # per-core degree (own dst half, incl self-loop) -> dinv
        deg = np.zeros(NP, np.float64)
        wl = all_wlists[c]
        for w in range(cfg.NW):
            for s in (0, 1):
                di = wl[w][s][1]
                if len(di):
                    np.add.at(deg, w * cfg.WIN + di, 1.0)
        dinv_own = np.where(deg > 0, 1.0 / np.sqrt(deg), 0.0).astype(np.float32)
        cores.append(dict(h=h, r=r, dinv_own=dinv_own, wl=wl))

    # full dinv per relation (both halves) for the H1 table scale
    for r in range(cfg.R):
        full = np.zeros(NPF, np.float32)
        full[0:NP] = cores[2 * r]["dinv_own"]
        full[NP:] = cores[2 * r + 1]["dinv_own"]
        for h in (0, 1):
            cores[2 * r + h]["dinv_full"] = full

    out_cores = []
    for c in range(2 * cfg.R):
        r, h = c // 2, c % 2
        cd = cores[c]
        gidx0, gidx1, drel = pack_core(cfg, cd["wl"])
        # batch split
        bm = (batch >= NH) == (h == 1)
        pos = np.nonzero(bm)[0]
        bl = (batch[bm] - h * NH).astype(np.int64)
        bidx = np.zeros(cfg.BP, np.int64)
        bidx[:len(bl)] = rowperm(bl)
        W1p = W1[r].reshape(F // 128, 128, H).transpose(1, 0, 2).copy()  # [128, F//128, H]
        W2p = np.zeros((H, 2 * OP), np.float32)
        W2p[:, :O] = W2[r]
        bn2g = np.zeros((OP, 1), np.float32); bn2g[:O, 0] = g2[r]
        bn2b = np.zeros((OP, 1), np.float32); bn2b[:O, 0] = be2[r]
        dinv_colF = cd["dinv_full"].reshape(cfg.NBLKF, 128).T.copy()   # [128, NBLKF]
        out_cores.append(dict(
            tensors=dict(
                fT=fT,                      # cast bf16 at feed
                W1=W1p,                     # cast bf16
                W2p=W2p,                    # cast bf16
                bn1g=g1[r].reshape(H, 1).copy(), bn1b=be1[r].reshape(H, 1).copy(),
                bn2g=bn2g, bn2b=bn2b,
                gidx0=gidx0, gidx1=gidx1,
                drel=drel,                  # cast bf16
                bidx=wrap_idx(bidx.astype(np.int16)),
                iota=iota,                  # cast bf16
                dinv_colF=dinv_colF,        # f32 [128, NBLKF]
                dinv_row=cd["dinv_own"].reshape(1, NP).copy(),  # f32
                ones_row=ones_row,
            ),
            pos=pos, nb=len(bl),
        ))
    return out_cores


import concourse.bass as bass  # noqa: E402,F401
import concourse.mybir as mybir  # noqa: E402
import concourse.tile as tile  # noqa: E402
import concourse.bacc as bacc  # noqa: E402

f32 = mybir.dt.float32
bf16 = mybir.dt.bfloat16
i16 = mybir.dt.int16
AF = mybir.ActivationFunctionType
ALU = mybir.AluOpType
EPS = 1e-5
SG = 16          # chunks per S supertile
NUM_Q = 3        # SWDGE queues (rotated per gather supertile)
SP = False       # single_packet for gathers


def build(cfg, n_cores=8):
    N, F, H, O, OP = cfg.N, cfg.F, cfg.H, cfg.O, cfg.OP
    NP, NPF, NW, WIN, BP = cfg.NP, cfg.NPF, cfg.NW, cfg.WIN, cfg.BP
    NBANK, BANKW, NBLK, NBLKF = cfg.NBANK, cfg.BANKW, cfg.NBLK, cfg.NBLKF
    NG, CH, LS = cfg.NG, cfg.CH, cfg.LS
    counts = cfg.counts
    CPC = NG // 128
    PAIRS = [[2 * i, 2 * i + 1] for i in range(max(1, n_cores // 2))]

    nc = bacc.Bacc(None, target_bir_lowering=False,
                   num_swdge_queues=NUM_Q)

    fTd = nc.dram_tensor("fT", [NPF // 512, 128, F // 128, 512], bf16, kind="ExternalInput")
    W1d = nc.dram_tensor("W1", [128, F // 128, H], bf16, kind="ExternalInput")
    W2d = nc.dram_tensor("W2p", [H, 2 * OP], bf16, kind="ExternalInput")
    bn1g = nc.dram_tensor("bn1g", [H, 1], f32, kind="ExternalInput")
    bn1b = nc.dram_tensor("bn1b", [H, 1], f32, kind="ExternalInput")
    bn2g = nc.dram_tensor("bn2g", [OP, 1], f32, kind="ExternalInput")
    bn2b = nc.dram_tensor("bn2b", [OP, 1], f32, kind="ExternalInput")
    gidxd = [nc.dram_tensor("gidx0", [128, LS[0] // 16], i16, kind="ExternalInput"),
             nc.dram_tensor("gidx1", [128, LS[1] // 16], i16, kind="ExternalInput")]
    dreld = nc.dram_tensor("drel", [128, CH], bf16, kind="ExternalInput")
    iotad = nc.dram_tensor("iota", [128, WIN], bf16, kind="ExternalInput")
    bidxd = nc.dram_tensor("bidx", [128, BP // 16], i16, kind="ExternalInput")
    dcolFd = nc.dram_tensor("dinv_colF", [128, NBLKF], f32, kind="ExternalInput")
    dcolOd = nc.dram_tensor("dinv_colO", [128, NBLK], f32, kind="ExternalInput")
    dpatd = nc.dram_tensor("dinv_pat", [128, NP], f32, kind="ExternalInput")
    identd = nc.dram_tensor("ident", [128, 128], f32, kind="ExternalInput")
    onesrd = nc.dram_tensor("ones_row", [1, 128], f32, kind="ExternalInput")
    outd = nc.dram_tensor("out", [BP, OP], f32, kind="ExternalOutput")

    # per-window chunk walk metadata (shared across cores)
    # wchunks[w] = list of (s, global_ci)
    wchunks = []
    ci = 0
    for w in range(NW):
        lst = []
        for s in (0, 1):
            for _ in range(counts[w][s]):
                lst.append((s, ci))
                ci += 1
        wchunks.append(lst)
    assert ci == CH

    with tile.TileContext(nc) as tc:
        with (
            tc.tile_pool(name="const", bufs=1) as cp,
            tc.tile_pool(name="dram", bufs=1, space="DRAM") as dp,
            tc.tile_pool(name="gbufp", bufs=1) as gp,
        ):
            ident = cp.tile([128, 128], f32); nc.sync.dma_start(ident[:], identd[:])
            ones_row = cp.tile([1, 128], f32); nc.sync.dma_start(ones_row[:], onesrd[:])
            iota = cp.tile([128, WIN], bf16); nc.sync.dma_start(iota[:], iotad[:])
            drel = cp.tile([128, CH], bf16); nc.sync.dma_start(drel[:], dreld[:])
            W1sb = cp.tile([128, F // 128, H], bf16); nc.sync.dma_start(W1sb[:], W1d[:])
            W2sb = cp.tile([H, 2 * OP], bf16); nc.sync.dma_start(W2sb[:], W2d[:])
            dcolF = cp.tile([128, NBLKF], f32); nc.sync.dma_start(dcolF[:], dcolFd[:])
            dcolO = cp.tile([128, NBLK], f32); nc.sync.dma_start(dcolO[:], dcolOd[:])
            bn1gt = cp.tile([H, 1], f32); nc.sync.dma_start(bn1gt[:], bn1g[:])
            bn1bt = cp.tile([H, 1], f32); nc.sync.dma_start(bn1bt[:], bn1b[:])
            bn2gt = cp.tile([OP, 1], f32); nc.sync.dma_start(bn2gt[:], bn2g[:])
            bn2bt = cp.tile([OP, 1], f32); nc.sync.dma_start(bn2bt[:], bn2b[:])
            epst = cp.tile([128, 1], f32); nc.vector.memset(epst[:], EPS)
            st1 = cp.tile([H, 2 * NBANK], f32)
            st2 = cp.tile([OP, 2 * NBANK], f32)
            gbuf = gp.tile([128, NP], bf16)

            g1tab = dp.tile([NPF, H], bf16)
            g2half = dp.tile([NP, OP], f32)
            g2tab = dp.tile([2, NP, OP], f32)
            x2pre = dp.tile([NP, OP], f32)
            st1l = dp.tile([H, 2], f32)
            st1gl = dp.tile([H, 2], f32)
            st2l = dp.tile([OP, 2], f32)
            st2gl = dp.tile([OP, 2], f32)

            # ============ H1: full table h1*dinv, node-major ============
            with (
                tc.tile_pool(name="h1sb", bufs=4) as hp,
                tc.tile_pool(name="h1ps", bufs=4, space="PSUM") as hps,
            ):
                with nc.named_scope("h1"):
                    for g in range(NBLKF // 4):
                        ftile = hp.tile([128, F // 128, 512], bf16, tag="ftile")
                        nc.sync.dma_start(ftile[:], fTd[g])
                        h1p = hps.tile([128, 4, H], f32, tag="h1p")
                        for q in range(4):
                            for k in range(F // 128):
                                nc.tensor.matmul(
                                    h1p[:, q, :],
                                    ftile[:, k, 128 * q:128 * q + 128],
                                    W1sb[:, k, :],
                                    start=(k == 0), stop=(k == F // 128 - 1))
                        h1nm = hp.tile([128, 4, H], bf16, tag="h1nm")
                        for q in range(4):
                            nc.scalar.activation(
                                h1nm[:, q, :], h1p[:, q, :], AF.Copy,
                                scale=dcolF[:, 4 * g + q:4 * g + q + 1])
                        nc.sync.dma_start(
                            g1tab[512 * g:512 * g + 512, :].rearrange("(p q) h -> p q h", q=4),
                            h1nm[:])

            # ============ shared seg pass ============
            def seg_pass(scope, tab_views, elem, fin):
                with (
                    tc.tile_pool(name="segsb", bufs=4) as sp,
                    tc.tile_pool(name="finp", bufs=2) as fnp,
                    tc.tile_pool(name="gat", bufs=3) as gpp,
                    tc.tile_pool(name="idxp", bufs=2) as ip,
                    tc.tile_pool(name="patp", bufs=2) as pp2,
                    tc.tile_pool(name="psseg", bufs=3, space="PSUM") as pss,
                    tc.tile_pool(name="psfin", bufs=2, space="PSUM") as psf,
                ):
                    with nc.named_scope(scope):
                        pos = [0, 0]
                        cache = [dict(), dict()]
                        qctr = [0]
                        Ssup, g0 = None, -1

                        def payload(s):
                            q, slot = pos[s] // CPC, pos[s] % CPC
                            if q not in cache[s]:
                                idxt = ip.tile([128, NG // 16], i16, tag=f"idx{s}")
                                nc.sync.dma_start(
                                    idxt[:],
                                    gidxd[s][:, q * (NG // 16):(q + 1) * (NG // 16)])
                                gt = gpp.tile([128, CPC, elem], bf16, tag=f"gt{s}")
                                nc.gpsimd.dma_gather(
                                    gt[:], tab_views[s], idxt[:], NG, NG, elem,
                                    single_packet=SP, queue_num=qctr[0] % NUM_Q)
                                qctr[0] += 1
                                cache[s] = {q: gt}
                            pos[s] += 1
                            return cache[s][q][:, slot, :]

                        for b in range(NBANK):
                            segp = pss.tile([128, 512], f32, tag="segp")
                            patf = pp2.tile([128, 512], f32, tag="patf")
                            nc.sync.dma_start(patf[:], dpatd[:, 512 * b:512 * b + 512])
                            for j in range(BANKW):
                                w = b * BANKW + j
                                ck = wchunks[w]
                                for t, (s, ci) in enumerate(ck):
                                    if ci % SG == 0:
                                        g0 = ci
                                        Ssup = sp.tile([128, SG, WIN], bf16, tag="S")
                                        nsg = min(SG, CH - g0)
                                        nc.vector.tensor_tensor(
                                            Ssup[:, 0:nsg, :],
                                            drel[:, g0:g0 + nsg].unsqueeze(2)
                                                .broadcast_to([128, nsg, WIN]),
                                            iota[:].unsqueeze(1)
                                                .broadcast_to([128, nsg, WIN]),
                                            ALU.is_equal)
                                    pl = payload(s)
                                    nc.tensor.matmul(
                                        segp[:, WIN * j:WIN * j + WIN],
                                        pl, Ssup[:, ci - g0, :],
                                        start=(t == 0), stop=(t == len(ck) - 1))
                            fin(b, segp, patf, dict(sb=fnp, ps=psf))

            # ============ L1 fin ============
            def fin1(b, segp, patf, pools):
                sl = gbuf[:, 512 * b:512 * b + 512]
                nc.vector.tensor_tensor(sl, segp[:], patf[:], ALU.mult)
                nc.vector.tensor_reduce(
                    st1[:, 2 * b:2 * b + 1], sl, mybir.AxisListType.X, ALU.add)
                scr = pools["sb"].tile([128, 512], f32, tag="scr")
                nc.scalar.activation(scr[:], sl, AF.Square)
                nc.vector.tensor_reduce(
                    st1[:, 2 * b + 1:2 * b + 2], scr[:], mybir.AxisListType.X, ALU.add)

            seg_pass("seg1", [g1tab[0:NP], g1tab[NP:NPF]], H, fin1)

            # ============ BN1 ============
            def bn_block(st, n_bank, stl, stgl, gt, bt, P, scope):
                with tc.tile_pool(name=f"bn{scope}", bufs=1) as bp:
                    stv = bp.tile([P, 2], f32)
                    nc.vector.tensor_reduce(
                        stv[:, 0:1], st[:, 0:2 * n_bank:2], mybir.AxisListType.X, ALU.add)
                    nc.vector.tensor_reduce(
                        stv[:, 1:2], st[:, 1:2 * n_bank:2], mybir.AxisListType.X, ALU.add)
                    nc.sync.dma_start(stl[:], stv[:])
                    nc.gpsimd.collective_compute(
                        "AllReduce", ALU.add, replica_groups=PAIRS,
                        ins=[stl.opt()], outs=[stgl.opt()])
                    stg = bp.tile([P, 2], f32)
                    nc.sync.dma_start(stg[:], stgl[:])
                    mean = bp.tile([P, 1], f32)
                    nc.vector.tensor_scalar_mul(mean[:], stg[:, 0:1], 1.0 / N)
                    var = bp.tile([P, 1], f32)
                    nc.vector.tensor_scalar_mul(var[:], stg[:, 1:2], 1.0 / N)
                    msq = bp.tile([P, 1], f32)
                    nc.vector.tensor_tensor(msq[:], mean[:], mean[:], ALU.mult)
                    nc.vector.tensor_tensor(var[:], var[:], msq[:], ALU.subtract)
                    nc.scalar.activation(var[:], var[:], AF.Sqrt, bias=epst[0:P, :])
                    rstd = bp.tile([P, 1], f32)
                    nc.vector.reciprocal(rstd[:], var[:])
                    sc = bp.tile([P, 1], f32)
                    nc.vector.tensor_tensor(sc[:], gt[:], rstd[:], ALU.mult)
                    sh = bp.tile([P, 1], f32)
                    nc.vector.tensor_tensor(sh[:], mean[:], sc[:], ALU.mult)
                    nc.vector.tensor_tensor(sh[:], bt[:], sh[:], ALU.subtract)
                    return sc, sh

            with nc.named_scope("bn1"):
                sc1, sh1 = bn_block(st1, NBANK, st1l, st1gl, bn1gt, bn1bt, H, "1")
                nc.scalar.activation(gbuf[:], gbuf[:], AF.Relu, bias=sh1[:], scale=sc1[:])

            # ============ L2 table ============
            with (
                tc.tile_pool(name="l2sb", bufs=3) as lp,
                tc.tile_pool(name="l2ps", bufs=3, space="PSUM") as lps,
            ):
                with nc.named_scope("l2tab"):
                    for g in range(NBLK // 4):
                        h2p = lps.tile([128, 4, 2 * OP], f32, tag="h2p")
                        for q in range(4):
                            t = 4 * g + q
                            nc.tensor.matmul(
                                h2p[:, q, :], gbuf[:, 128 * t:128 * t + 128],
                                W2sb[:], start=True, stop=True)
                        h2nm = lp.tile([128, 4, 2 * OP], bf16, tag="h2nm")
                        for q in range(4):
                            nc.scalar.activation(
                                h2nm[:, q, :], h2p[:, q, :], AF.Copy,
                                scale=dcolO[:, 4 * g + q:4 * g + q + 1])
                        nc.sync.dma_start(
                            g2half[512 * g:512 * g + 512, :].rearrange("(p q) e -> p q e", q=4),
                            h2nm[:].bitcast(f32))

            nc.gpsimd.collective_compute(
                "AllGather", ALU.bypass, replica_groups=PAIRS,
                ins=[g2half.opt()], outs=[g2tab.opt()])

            # ============ L2 fin ============
            def fin2(b, segp, patf, pools):
                x2sl = pools["sb"].tile([64, 512], f32, tag="x2sl")
                nc.vector.tensor_tensor(x2sl[:], segp[0:64, :], patf[0:64, :], ALU.mult)
                nc.vector.tensor_reduce(
                    st2[:, 2 * b:2 * b + 1], x2sl[:], mybir.AxisListType.X, ALU.add)
                scr = pools["sb"].tile([64, 512], f32, tag="scr2")
                nc.scalar.activation(scr[:], x2sl[:], AF.Square)
                nc.vector.tensor_reduce(
                    st2[:, 2 * b + 1:2 * b + 2], scr[:], mybir.AxisListType.X, ALU.add)
                x2nm = pools["sb"].tile([128, 4, OP], f32, tag="x2nm")
                for q in range(4):
                    trp = pools["ps"].tile([128, 128], f32, tag="trp")
                    nc.tensor.transpose(trp[0:128, 0:64], x2sl[:, 128 * q:128 * q + 128],
                                        ident[0:64, 0:64])
                    nc.scalar.activation(x2nm[:, q, :], trp[0:128, 0:64], AF.Copy)
                nc.sync.dma_start(
                    x2pre[512 * b:512 * b + 512, :].rearrange("(p q) e -> p q e", q=4),
                    x2nm[:])

            seg_pass("seg2", [g2tab[0].bitcast(bf16), g2tab[1].bitcast(bf16)], 2 * OP, fin2)

            # ============ BN2 + FINAL ============
            with (
                tc.tile_pool(name="fsb", bufs=2) as fp2,
                tc.tile_pool(name="fps", bufs=2, space="PSUM") as pf,
            ):
                with nc.named_scope("final"):
                    bidxt0 = fp2.tile([128, BP // 16], i16, tag="bidx")
                    nc.sync.dma_start(bidxt0[:], bidxd[:])
                    NBB0 = BP // 128
                    xb0 = fp2.tile([128, NBB0, OP], f32, tag="xb")
                    nc.gpsimd.dma_gather(xb0[:], x2pre[:], bidxt0[:], BP, BP, OP,
                                         single_packet=False)
                    sc2, sh2 = bn_block(st2, NBANK, st2l, st2gl, bn2gt, bn2bt, OP, "2")
                    reps = []
                    for vi, v in enumerate((sc2, sh2)):
                        rowp = pf.tile([128, 128], f32, tag="rowp")
                        nc.tensor.matmul(rowp[0:1, 0:OP], v[:], ident[0:OP, 0:OP],
                                         start=True, stop=True)
                        rowsb = fp2.tile([1, OP], f32, tag=f"rowsb{vi}")
                        nc.scalar.activation(rowsb[:], rowp[0:1, 0:OP], AF.Copy)
                        repp = pf.tile([128, OP], f32, tag=f"rep{vi}")
                        nc.tensor.matmul(repp[:], ones_row[:], rowsb[:], start=True, stop=True)
                        rep = fp2.tile([128, OP], f32, tag=f"repsb{vi}")
                        nc.vector.tensor_copy(rep[:], repp[:])
                        reps.append(rep)
                    sc_rep, sh_rep = reps

                    NBB = NBB0
                    xb = xb0
                    nc.vector.tensor_tensor(
                        xb[:], xb[:], sc_rep[:].unsqueeze(1).broadcast_to([128, NBB, OP]),
                        ALU.mult)
                    nc.vector.tensor_tensor(
                        xb[:], xb[:], sh_rep[:].unsqueeze(1).broadcast_to([128, NBB, OP]),
                        ALU.add)
                    nc.scalar.activation(xb[:], xb[:], AF.Relu)
                    xs = xb[:, :, 0:O]
                    mx = fp2.tile([128, NBB, 1], f32, tag="mx")
                    nc.vector.tensor_reduce(mx[:], xs, mybir.AxisListType.X, ALU.max)
                    nc.vector.tensor_tensor(xs, xs, mx[:].broadcast_to([128, NBB, O]),
                                            ALU.subtract)
                    ex = fp2.tile([128, NBB, O], f32, tag="ex")
                    nc.scalar.activation(ex[:], xs, AF.Exp)
                    sm = fp2.tile([128, NBB, 1], f32, tag="sm")
                    nc.vector.tensor_reduce(sm[:], ex[:], mybir.AxisListType.X, ALU.add)
                    nc.scalar.activation(sm[:], sm[:], AF.Ln)
                    nc.vector.tensor_tensor(xs, xs, sm[:].broadcast_to([128, NBB, O]),
                                            ALU.subtract)
                    nc.sync.dma_start(outd[:].rearrange("(g p) e -> p g e", p=128), xb[:])

    nc.compile()
    return nc


import ml_dtypes  # noqa: E402

TRACE = False
LAST = {"exec_time_ns": None}
_CACHE = {}


def _get_program(cfg):
    key = (cfg.N, cfg.F, cfg.H, cfg.O, cfg.R, cfg.E, cfg.B, hash(cfg.counts))
    if key not in _CACHE:
        _CACHE[key] = build(cfg, n_cores=8)
    return _CACHE[key]


def kernel(**inputs):
    from concourse.bass_utils import run_bass_kernel_spmd

    inputs = {k: np.asarray(v) for k, v in inputs.items()}
    N, F = inputs["features"].shape
    R, _, E = inputs["multi_r_edge_index"].shape
    B = inputs["batch_nodes"].shape[0]
    H = inputs["W1"].shape[2]
    O = inputs["W2"].shape[2]
    cfg = Cfg(N=N, F=F, H=H, O=O, R=R, E=E, B=B)
    cores = prep_all(cfg, inputs)
    nc = _get_program(cfg)

    ident = np.eye(128, dtype=np.float32)

    in_maps = []
    for c in range(2 * R):
        t = cores[c]["tensors"]
        dpat = np.tile(t["dinv_row"], (128, 1)).astype(np.float32)
        in_maps.append(dict(
            fT=t["fT"].astype(ml_dtypes.bfloat16),
            W1=t["W1"].astype(ml_dtypes.bfloat16),
            W2p=t["W2p"].astype(ml_dtypes.bfloat16),
            bn1g=t["bn1g"], bn1b=t["bn1b"], bn2g=t["bn2g"], bn2b=t["bn2b"],
            gidx0=t["gidx0"], gidx1=t["gidx1"],
            drel=t["drel"].astype(ml_dtypes.bfloat16),
            bidx=t["bidx"],
            iota=t["iota"].astype(ml_dtypes.bfloat16),
            dinv_colF=t["dinv_colF"],
            dinv_colO=t["dinv_colF"][:, (c % 2) * cfg.NBLK:(c % 2 + 1) * cfg.NBLK].copy(),
            dinv_pat=dpat,
            ident=ident,
            ones_row=t["ones_row"],
        ))

    res = run_bass_kernel_spmd(nc, in_maps, core_ids=list(range(2 * R)), trace=TRACE)
    LAST["exec_time_ns"] = res.exec_time_ns
    LAST["results"] = res

    out = np.zeros((B, R * O), np.float32)
    for c in range(2 * R):
        core = cores[c]
        r = c // 2
        row = np.asarray(res.results[c]["out"])
        out[core["pos"], r * O:(r + 1) * O] = row[:core["nb"], :O]
    return out

